# revision 27
# baseline (speedup 1.0000x reference)
"""Trainium2 Bass kernel for ragged bmm2 (attention probs @ V, grouped GEMM).

Problem: 32 ragged sequences, lengths s_i = 128 + 12*i (128..500), 16 heads,
embed 64.  batch1 = packed per-(seq,head) [s,s] prob blocks (fp32, ~227MB),
batch2 = packed V [ntokens, 16*64].  out[q,h,e] = sum_k P[h,q,k] V[k,h,e].

Sharding: head-parallel.  Core c handles heads (2c, 2c+1) for ALL sequences:
identical per-core work/schedule (SPMD-friendly), perfect balance.

v2 design (vs v1): all compute in fp16; P is transposed on the HOST during
packing so the device does zero PE transposes; the matmul streams P^T as the
moving operand against a stationary V chunk, producing out^T [64, s] blocks
per (seq, head) with only n_k matmul instructions per (seq, head); out^T is
un-transposed on the host.  All HBM buffers are packed partition-major so
every DMA is long contiguous runs per partition.

Device per (seq): DMA P^T tiles for both heads, then per (head, k-chunk) one
matmul acc[64h:64h+64, 0:s] += V_chunk[kn,64].T @ PT_chunk[kn, s] accumulated
over k-chunks in PSUM, copy [128, s] fp32->fp16 to an SBUF staging tile, and
one store DMA per group of sequences.
"""

import math

import numpy as np

import bass_rust
import concourse.bass as bass
import concourse.tile as tile
import concourse.mybir as mybir
from concourse.vector_clock import ScopedClock
from concourse.bass2jax import install_neuronx_cc_hook, _bass_exec_p

# ---------------------------------------------------------------------------
# Workarounds for the in-container walrus build, which only accepts a small
# number of sem waits per instruction: split excess waits onto NoOps placed
# immediately before the instruction on the same engine queue.
# ---------------------------------------------------------------------------
MAX_WAITS = 1

_nop_ctr = [0]


def _mk_wait_nop(engine, waits):
    _nop_ctr[0] += 1
    nop = bass_rust.InstNoOp(name=f"I-waitsplit-{_nop_ctr[0]}", ins=[], outs=[],
                             engine=engine)
    nop.sync_info = bass_rust.SyncInfo(on_wait=list(waits), on_update=[])
    return nop


def _split_inst_waits(ordered):
    for bb_name, insts in ordered.items():
        new = []
        for inst in insts:
            si = getattr(inst, "sync_info", None)
            eng = getattr(inst, "engine", None)
            if si is not None and eng is not None:
                waits = list(si.on_wait)
                if len(waits) > MAX_WAITS:
                    extra, keep = waits[:-MAX_WAITS], waits[-MAX_WAITS:]
                    for j in range(0, len(extra), MAX_WAITS):
                        new.append(_mk_wait_nop(eng, extra[j:j + MAX_WAITS]))
                    inst.sync_info = bass_rust.SyncInfo(
                        on_wait=keep, on_update=list(si.on_update))
            new.append(inst)
        insts[:] = new
    return ordered


if not getattr(tile.TileContext, "_waitsplit_patched", False):
    _orig_lower = tile.TileContext._lower_ordered_insts

    def _patched_lower(self, ordered):
        return _orig_lower(self, _split_inst_waits(ordered))

    def _patched_drain_and_barrier(self, tick_clock, wait_clock):
        nc = self.nc
        drain_inst = nc.sync.drain()
        wait_clock.add_sem_waits(
            drain_inst.ins, ScopedClock({None: tick_clock.global_clock}))
        si = drain_inst.ins.sync_info
        waits = list(si.on_wait)
        if len(waits) > MAX_WAITS:
            drain_inst.ins.sync_info = bass_rust.SyncInfo(
                on_wait=waits[:MAX_WAITS], on_update=list(si.on_update))
            for j in range(MAX_WAITS, len(waits), MAX_WAITS):
                nop = nc.sync.nop(nofuse=True)
                nop.ins.sync_info = bass_rust.SyncInfo(
                    on_wait=waits[j:j + MAX_WAITS], on_update=[])
        nc.all_engine_barrier()
        assert self.sems is not None
        popped = nc._tile_sem_poison_stack.pop()
        assert popped is self._sem_poison
        nc.clear_and_free_semaphores(list(self.sems.allocated().values()))
        nc.all_engine_barrier()

    tile.TileContext._lower_ordered_insts = _patched_lower
    tile.TileContext._drain_and_barrier = _patched_drain_and_barrier
    tile.TileContext._waitsplit_patched = True

HEADS = 16
EMBED = 64
BATCH = 32
N_CORES = 8
P = 128  # partitions

SEQS = [128 + 12 * i for i in range(BATCH)]
NTOK = sum(SEQS)  # 10048
# seq start offsets in batch1 (elements) and batch2 (rows)
_A = np.concatenate([[0], np.cumsum([HEADS * s * s for s in SEQS])])
_B = np.concatenate([[0], np.cumsum(SEQS)])
# schedule: a few shortest first (fast pipeline ramp: the first matmul only
# waits on tiny V/P transfers), then the rest in descending length (ends
# with small tail)
ORDER = [0, 1, 2, 3] + sorted(range(4, BATCH), key=lambda i: -SEQS[i])

_NF = {i: SEQS[i] // P for i in range(BATCH)}           # full k-chunks
_REM = {i: SEQS[i] % P for i in range(BATCH)}           # remainder k rows
_NK = {i: math.ceil(SEQS[i] / P) for i in range(BATCH)}  # total k-chunks

# ---- packed P^T layout: global [128, PCOLS] partition-major -------------
# P^T chunks are grouped into DMA clusters (defined below); each cluster
# occupies a contiguous col span [A region: full 128-row chunks][B region:
# remainder chunks, rows 0..rem].  Partition p = k row within chunk.

# ---- packed V layout: global [128, VCOLS]; per seq 128-row-aligned ------
# chunk kc of seq i at cols (VOFF[i]+kc)*128 .. +128 (cols = head0|head1
# embed, 2*EMBED=128 wide); partition = k row within chunk (zero padded).
_VOFF = {}
_vc = 0
for _i in ORDER:
    _VOFF[_i] = _vc
    _vc += _NK[_i]
VCOLS = _vc * (2 * EMBED)  # 94*128 = 12032

# ---- out^T layout: global [128, OUTCOLS=NTOK] -----------------------------
# per seq block [128, s]: partitions 0:64 = head even out^T [64, s],
# 64:128 = head odd.
_OOFF = {}
_oc = 0
for _i in ORDER:
    _OOFF[_i] = _oc
    _oc += SEQS[_i]
OUTCOLS = _oc  # 10048

# store groups: consecutive runs of ORDER staged in one SBUF tile + 1 DMA
_GSIZES = [4, 8, 8, 8, 4]
GROUPS = []
_g0 = 0
for _gs in _GSIZES:
    GROUPS.append(ORDER[_g0:_g0 + _gs])
    _g0 += _gs
assert _g0 == BATCH

# Per-seq P tile layout [128, 2*n_k*s]: region A = full k-chunks of both
# heads (cols (hh*nf+kc)*s, 128 rows), then region B = remainder chunks
# (cols 2*nf*s + hh*s, rows 0..rem).  A and B are DMAed separately (B with
# rem partition rows => exact transfer bytes), on different issue engines.
_PAOFF = {}  # HBM col offset of seq's region A
_pc = 0
for _i in ORDER:
    _PAOFF[_i] = _pc
    _pc += 2 * _NK[_i] * SEQS[_i]
PCOLS = _pc  # 2*sum(n_k*s) = 64816

COMPUTE_DT = mybir.dt.float16
_NP_DT = np.float16


def build_program(repeat: int = 1):
    """Build the Bass program (one SPMD program shared by all 8 cores)."""
    nc = bass.Bass("TRN2", target_bir_lowering=False, debug=False,
                   num_devices=N_CORES)
    cdt = COMPUTE_DT
    p_d = nc.dram_tensor("p", [P, PCOLS], cdt, kind="ExternalInput").ap()
    v_d = nc.dram_tensor("v", [P, VCOLS], cdt, kind="ExternalInput").ap()
    o_d = nc.dram_tensor("o", [P, OUTCOLS], cdt, kind="ExternalOutput").ap()

    with tile.TileContext(nc) as tc:
        with (
            tc.tile_pool(name="vpool", bufs=len(GROUPS)) as vpool,
            tc.tile_pool(name="ppool", bufs=10) as ppool,
            tc.tile_pool(name="accp", bufs=8, space="PSUM") as acc_pool,
            tc.tile_pool(name="outsb", bufs=2) as out_pool,
        ):
            # V stays resident in SBUF, one tile per store-group so matmuls
            # only depend on their own group's V DMA
            _vgrp = []
            _c0 = 0
            for grp in GROUPS:
                last = grp[-1]
                _c1 = (_VOFF[last] + _NK[last]) * 2 * EMBED
                _vgrp.append((_c0, _c1))
                _c0 = _c1
            vts = {}
            for g in range(len(GROUPS)):
                c0, c1 = _vgrp[g]
                vts[g] = vpool.tile([P, c1 - c0], cdt, name=f"vt{g}", tag="vt")
            nc.sync.dma_start(vts[0][:], v_d[:, _vgrp[0][0]:_vgrp[0][1]])

            for _rep in range(repeat):
              flip = 0
              for g, grp in enumerate(GROUPS):
                vt = vts[g]
                vbase = _vgrp[g][0]
                gbase = _OOFF[grp[0]]
                gcols = sum(SEQS[i] for i in grp)
                osb = out_pool.tile([P, gcols], cdt, tag="osb")
                for si, i in enumerate(grp):
                    if si == 2 and g + 1 < len(GROUPS):
                        c0, c1 = _vgrp[g + 1]
                        nc.scalar.dma_start(vts[g + 1][:], v_d[:, c0:c1])
                    s = SEQS[i]
                    nf, rem, n_k = _NF[i], _REM[i], _NK[i]
                    pt = ppool.tile([P, 2 * n_k * s], cdt,
                                    name=f"pt{i}", tag="pt")
                    poff = _PAOFF[i]
                    ca = 2 * nf * s
                    e0, e1 = ((nc.sync, nc.scalar) if flip == 0
                              else (nc.scalar, nc.sync))
                    flip ^= 1
                    e0.dma_start(pt[:, 0:ca], p_d[:, poff:poff + ca])
                    if rem:
                        e1.dma_start(
                            pt[0:rem, ca:ca + 2 * s],
                            p_d[0:rem, poff + ca:poff + ca + 2 * s])
                    acc = acc_pool.tile([P, s], mybir.dt.float32,
                                        name=f"acc{i}", tag="acc")
                    for hh in range(2):
                        for kc in range(n_k):
                            kn = P if kc < nf else rem
                            vcol = ((_VOFF[i] + kc) * 2 * EMBED
                                    + hh * EMBED - vbase)
                            if kc < nf:
                                pcol = (hh * nf + kc) * s
                            else:
                                pcol = ca + hh * s
                            nc.tensor.matmul(
                                acc[hh * EMBED:(hh + 1) * EMBED, 0:s],
                                lhsT=vt[0:kn, vcol:vcol + EMBED],
                                rhs=pt[0:kn, pcol:pcol + s],
                                start=(kc == 0),
                                stop=(kc == n_k - 1),
                            )
                    ocol = _OOFF[i] - gbase
                    nc.vector.tensor_copy(osb[:, ocol:ocol + s], acc[:])
                seng = nc.scalar if g % 2 == 0 else nc.sync
                seng.dma_start(o_d[:, gbase:gbase + gcols], osb[:])
    return nc


def pack_inputs(batch1: np.ndarray, batch2: np.ndarray):
    """Build per-core packed (p_core [128, PCOLS], v_core [128, VCOLS])."""
    b2 = np.ascontiguousarray(batch2).reshape(NTOK, HEADS * EMBED)
    p_cores = []
    v_cores = []
    for c in range(N_CORES):
        pc = np.zeros((P, PCOLS), dtype=_NP_DT)
        vc = np.zeros((P, VCOLS), dtype=_NP_DT)
        for i in ORDER:
            s = SEQS[i]
            nf, rem, n_k = _NF[i], _REM[i], _NK[i]
            aoff = _PAOFF[i]
            boff = aoff + 2 * nf * s
            for hh in range(2):
                h = 2 * c + hh
                blk = batch1[_A[i] + h * s * s: _A[i] + (h + 1) * s * s]
                blkT = blk.reshape(s, s).T.astype(_NP_DT)  # [k, q]
                if nf:
                    po = aoff + hh * nf * s
                    pc[:, po:po + nf * s] = (
                        blkT[:nf * P].reshape(nf, P, s)
                        .transpose(1, 0, 2).reshape(P, nf * s))
                if rem:
                    po = boff + hh * s
                    pc[0:rem, po:po + s] = blkT[nf * P:]
            # V rows for this seq, both local heads, 128-aligned chunks
            vrows = b2[_B[i]:_B[i] + s, 2 * c * EMBED:(2 * c + 2) * EMBED]
            vcol = _VOFF[i] * 2 * EMBED
            vpad = np.zeros((n_k * P, 2 * EMBED), dtype=_NP_DT)
            vpad[:s] = vrows.astype(_NP_DT)
            vc[:, vcol:vcol + n_k * 2 * EMBED] = (
                vpad.reshape(n_k, P, 2 * EMBED)
                .transpose(1, 0, 2).reshape(P, n_k * 2 * EMBED))
        p_cores.append(pc)
        v_cores.append(vc)
    return p_cores, v_cores


def unpack_outputs(o_cores) -> np.ndarray:
    """Scatter per-core packed out^T back to [NTOK, HEADS, EMBED] fp32."""
    out = np.empty((NTOK, HEADS * EMBED), dtype=np.float32)
    for c in range(N_CORES):
        oc = np.asarray(o_cores[c], dtype=np.float32)
        for i in ORDER:
            s = SEQS[i]
            blk = oc[:, _OOFF[i]:_OOFF[i] + s]  # [128, s]
            out[_B[i]:_B[i] + s, 2 * c * EMBED:(2 * c + 1) * EMBED] = \
                blk[0:EMBED].T
            out[_B[i]:_B[i] + s, (2 * c + 1) * EMBED:(2 * c + 2) * EMBED] = \
                blk[EMBED:2 * EMBED].T
    return out.reshape(NTOK, HEADS, EMBED)


# ---------------------------------------------------------------------------
# Execution: run_bass_kernel_spmd over 8 cores (axon/PJRT path).
# ---------------------------------------------------------------------------
_CACHE = {}


def make_in_maps(batch1, batch2):
    p_cores, v_cores = pack_inputs(
        np.asarray(batch1, np.float32), np.asarray(batch2, np.float32))
    return [{"p": p_cores[c], "v": v_cores[c]} for c in range(N_CORES)]


def run_packed(in_maps):
    """Run the SPMD program; returns list of per-core packed outputs."""
    import concourse.bass_utils as bass_utils

    if ("nc", 1) not in _CACHE:
        _CACHE[("nc", 1)] = build_program()
    nc = _CACHE[("nc", 1)]
    res = bass_utils.run_bass_kernel_spmd(nc, in_maps,
                                          core_ids=list(range(N_CORES)))
    return [res.results[c]["o"] for c in range(N_CORES)]


def kernel(batch1, batch2, batch, seqlen) -> np.ndarray:
    in_maps = make_in_maps(batch1, batch2)
    o_cores = run_packed(in_maps)
    return unpack_outputs(o_cores)


# revision 29
# speedup vs baseline: 1.0050x; 1.0050x over previous
"""Trainium2 Bass kernel for ragged bmm2 (attention probs @ V, grouped GEMM).

Problem: 32 ragged sequences, lengths s_i = 128 + 12*i (128..500), 16 heads,
embed 64.  batch1 = packed per-(seq,head) [s,s] prob blocks (fp32, ~227MB),
batch2 = packed V [ntokens, 16*64].  out[q,h,e] = sum_k P[h,q,k] V[k,h,e].

Sharding: head-parallel.  Core c handles heads (2c, 2c+1) for ALL sequences:
identical per-core work/schedule (SPMD-friendly), perfect balance.

v2 design (vs v1): all compute in fp16; P is transposed on the HOST during
packing so the device does zero PE transposes; the matmul streams P^T as the
moving operand against a stationary V chunk, producing out^T [64, s] blocks
per (seq, head) with only n_k matmul instructions per (seq, head); out^T is
un-transposed on the host.  All HBM buffers are packed partition-major so
every DMA is long contiguous runs per partition.

Device per (seq): DMA P^T tiles for both heads, then per (head, k-chunk) one
matmul acc[64h:64h+64, 0:s] += V_chunk[kn,64].T @ PT_chunk[kn, s] accumulated
over k-chunks in PSUM, copy [128, s] fp32->fp16 to an SBUF staging tile, and
one store DMA per group of sequences.
"""

import math

import numpy as np

import bass_rust
import concourse.bass as bass
import concourse.tile as tile
import concourse.mybir as mybir
from concourse.vector_clock import ScopedClock
from concourse.bass2jax import install_neuronx_cc_hook, _bass_exec_p

# ---------------------------------------------------------------------------
# Workarounds for the in-container walrus build, which only accepts a small
# number of sem waits per instruction: split excess waits onto NoOps placed
# immediately before the instruction on the same engine queue.
# ---------------------------------------------------------------------------
MAX_WAITS = 1

_nop_ctr = [0]


def _mk_wait_nop(engine, waits):
    _nop_ctr[0] += 1
    nop = bass_rust.InstNoOp(name=f"I-waitsplit-{_nop_ctr[0]}", ins=[], outs=[],
                             engine=engine)
    nop.sync_info = bass_rust.SyncInfo(on_wait=list(waits), on_update=[])
    return nop


def _split_inst_waits(ordered):
    for bb_name, insts in ordered.items():
        new = []
        for inst in insts:
            si = getattr(inst, "sync_info", None)
            eng = getattr(inst, "engine", None)
            if si is not None and eng is not None:
                waits = list(si.on_wait)
                if len(waits) > MAX_WAITS:
                    extra, keep = waits[:-MAX_WAITS], waits[-MAX_WAITS:]
                    for j in range(0, len(extra), MAX_WAITS):
                        new.append(_mk_wait_nop(eng, extra[j:j + MAX_WAITS]))
                    inst.sync_info = bass_rust.SyncInfo(
                        on_wait=keep, on_update=list(si.on_update))
            new.append(inst)
        insts[:] = new
    return ordered


if not getattr(tile.TileContext, "_waitsplit_patched", False):
    _orig_lower = tile.TileContext._lower_ordered_insts

    def _patched_lower(self, ordered):
        return _orig_lower(self, _split_inst_waits(ordered))

    def _patched_drain_and_barrier(self, tick_clock, wait_clock):
        nc = self.nc
        drain_inst = nc.sync.drain()
        wait_clock.add_sem_waits(
            drain_inst.ins, ScopedClock({None: tick_clock.global_clock}))
        si = drain_inst.ins.sync_info
        waits = list(si.on_wait)
        if len(waits) > MAX_WAITS:
            drain_inst.ins.sync_info = bass_rust.SyncInfo(
                on_wait=waits[:MAX_WAITS], on_update=list(si.on_update))
            for j in range(MAX_WAITS, len(waits), MAX_WAITS):
                nop = nc.sync.nop(nofuse=True)
                nop.ins.sync_info = bass_rust.SyncInfo(
                    on_wait=waits[j:j + MAX_WAITS], on_update=[])
        nc.all_engine_barrier()
        assert self.sems is not None
        popped = nc._tile_sem_poison_stack.pop()
        assert popped is self._sem_poison
        nc.clear_and_free_semaphores(list(self.sems.allocated().values()))
        nc.all_engine_barrier()

    tile.TileContext._lower_ordered_insts = _patched_lower
    tile.TileContext._drain_and_barrier = _patched_drain_and_barrier
    tile.TileContext._waitsplit_patched = True

HEADS = 16
EMBED = 64
BATCH = 32
N_CORES = 8
P = 128  # partitions

SEQS = [128 + 12 * i for i in range(BATCH)]
NTOK = sum(SEQS)  # 10048
# seq start offsets in batch1 (elements) and batch2 (rows)
_A = np.concatenate([[0], np.cumsum([HEADS * s * s for s in SEQS])])
_B = np.concatenate([[0], np.cumsum(SEQS)])
# schedule: interleave shortest/longest so every pipeline window carries a
# balanced DMA-bytes vs PE-work mix; starts with the shortest seq (fast
# pipeline ramp: the first matmul only waits on tiny V/P transfers)
ORDER = []
for _a, _b in zip(range(BATCH // 2), range(BATCH - 1, BATCH // 2 - 1, -1)):
    ORDER += [_a, _b]

_NF = {i: SEQS[i] // P for i in range(BATCH)}           # full k-chunks
_REM = {i: SEQS[i] % P for i in range(BATCH)}           # remainder k rows
_NK = {i: math.ceil(SEQS[i] / P) for i in range(BATCH)}  # total k-chunks

# ---- packed P^T layout: global [128, PCOLS] partition-major -------------
# P^T chunks are grouped into DMA clusters (defined below); each cluster
# occupies a contiguous col span [A region: full 128-row chunks][B region:
# remainder chunks, rows 0..rem].  Partition p = k row within chunk.

# ---- packed V layout: global [128, VCOLS]; per seq 128-row-aligned ------
# chunk kc of seq i at cols (VOFF[i]+kc)*128 .. +128 (cols = head0|head1
# embed, 2*EMBED=128 wide); partition = k row within chunk (zero padded).
_VOFF = {}
_vc = 0
for _i in ORDER:
    _VOFF[_i] = _vc
    _vc += _NK[_i]
VCOLS = _vc * (2 * EMBED)  # 94*128 = 12032

# ---- out^T layout: global [128, OUTCOLS=NTOK] -----------------------------
# per seq block [128, s]: partitions 0:64 = head even out^T [64, s],
# 64:128 = head odd.
_OOFF = {}
_oc = 0
for _i in ORDER:
    _OOFF[_i] = _oc
    _oc += SEQS[_i]
OUTCOLS = _oc  # 10048

# store groups: consecutive runs of ORDER staged in one SBUF tile + 1 DMA
_GSIZES = [4, 8, 8, 8, 4]
GROUPS = []
_g0 = 0
for _gs in _GSIZES:
    GROUPS.append(ORDER[_g0:_g0 + _gs])
    _g0 += _gs
assert _g0 == BATCH

# Per-seq P tile layout [128, 2*n_k*s]: region A = full k-chunks of both
# heads (cols (hh*nf+kc)*s, 128 rows), then region B = remainder chunks
# (cols 2*nf*s + hh*s, rows 0..rem).  A and B are DMAed separately (B with
# rem partition rows => exact transfer bytes), on different issue engines.
_PAOFF = {}  # HBM col offset of seq's region A
_pc = 0
for _i in ORDER:
    _PAOFF[_i] = _pc
    _pc += 2 * _NK[_i] * SEQS[_i]
PCOLS = _pc  # 2*sum(n_k*s) = 64816

COMPUTE_DT = mybir.dt.float16
_NP_DT = np.float16


def build_program(repeat: int = 1):
    """Build the Bass program (one SPMD program shared by all 8 cores)."""
    nc = bass.Bass("TRN2", target_bir_lowering=False, debug=False,
                   num_devices=N_CORES)
    cdt = COMPUTE_DT
    p_d = nc.dram_tensor("p", [P, PCOLS], cdt, kind="ExternalInput").ap()
    v_d = nc.dram_tensor("v", [P, VCOLS], cdt, kind="ExternalInput").ap()
    o_d = nc.dram_tensor("o", [P, OUTCOLS], cdt, kind="ExternalOutput").ap()

    with tile.TileContext(nc) as tc:
        with (
            tc.tile_pool(name="vpool", bufs=len(GROUPS)) as vpool,
            tc.tile_pool(name="ppool", bufs=10) as ppool,
            tc.tile_pool(name="accp", bufs=8, space="PSUM") as acc_pool,
            tc.tile_pool(name="outsb", bufs=2) as out_pool,
        ):
            # V stays resident in SBUF, one tile per store-group so matmuls
            # only depend on their own group's V DMA
            _vgrp = []
            _c0 = 0
            for grp in GROUPS:
                last = grp[-1]
                _c1 = (_VOFF[last] + _NK[last]) * 2 * EMBED
                _vgrp.append((_c0, _c1))
                _c0 = _c1
            vts = {}
            for g in range(len(GROUPS)):
                c0, c1 = _vgrp[g]
                vts[g] = vpool.tile([P, c1 - c0], cdt, name=f"vt{g}", tag="vt")
            nc.sync.dma_start(vts[0][:], v_d[:, _vgrp[0][0]:_vgrp[0][1]])

            for _rep in range(repeat):
              flip = 0
              for g, grp in enumerate(GROUPS):
                vt = vts[g]
                vbase = _vgrp[g][0]
                gbase = _OOFF[grp[0]]
                gcols = sum(SEQS[i] for i in grp)
                osb = out_pool.tile([P, gcols], cdt, tag="osb")
                for si, i in enumerate(grp):
                    if si == 2 and g + 1 < len(GROUPS):
                        c0, c1 = _vgrp[g + 1]
                        nc.scalar.dma_start(vts[g + 1][:], v_d[:, c0:c1])
                    s = SEQS[i]
                    nf, rem, n_k = _NF[i], _REM[i], _NK[i]
                    pt = ppool.tile([P, 2 * n_k * s], cdt,
                                    name=f"pt{i}", tag="pt")
                    poff = _PAOFF[i]
                    ca = 2 * nf * s
                    e0, e1 = ((nc.sync, nc.scalar) if flip == 0
                              else (nc.scalar, nc.sync))
                    flip ^= 1
                    e0.dma_start(pt[:, 0:ca], p_d[:, poff:poff + ca])
                    if rem:
                        e1.dma_start(
                            pt[0:rem, ca:ca + 2 * s],
                            p_d[0:rem, poff + ca:poff + ca + 2 * s])
                    acc = acc_pool.tile([P, s], mybir.dt.float32,
                                        name=f"acc{i}", tag="acc")
                    for hh in range(2):
                        for kc in range(n_k):
                            kn = P if kc < nf else rem
                            vcol = ((_VOFF[i] + kc) * 2 * EMBED
                                    + hh * EMBED - vbase)
                            if kc < nf:
                                pcol = (hh * nf + kc) * s
                            else:
                                pcol = ca + hh * s
                            nc.tensor.matmul(
                                acc[hh * EMBED:(hh + 1) * EMBED, 0:s],
                                lhsT=vt[0:kn, vcol:vcol + EMBED],
                                rhs=pt[0:kn, pcol:pcol + s],
                                start=(kc == 0),
                                stop=(kc == n_k - 1),
                            )
                    ocol = _OOFF[i] - gbase
                    nc.vector.tensor_copy(osb[:, ocol:ocol + s], acc[:])
                    if g == len(GROUPS) - 1:
                        # last group: store per seq so the final transfer
                        # (and hence the kernel tail) is small
                        seng = nc.scalar if si % 2 == 0 else nc.sync
                        seng.dma_start(
                            o_d[:, _OOFF[i]:_OOFF[i] + s],
                            osb[:, ocol:ocol + s])
                if g < len(GROUPS) - 1:
                    seng = nc.scalar if g % 2 == 0 else nc.sync
                    seng.dma_start(o_d[:, gbase:gbase + gcols], osb[:])
    return nc


def pack_inputs(batch1: np.ndarray, batch2: np.ndarray):
    """Build per-core packed (p_core [128, PCOLS], v_core [128, VCOLS])."""
    b2 = np.ascontiguousarray(batch2).reshape(NTOK, HEADS * EMBED)
    p_cores = []
    v_cores = []
    for c in range(N_CORES):
        pc = np.zeros((P, PCOLS), dtype=_NP_DT)
        vc = np.zeros((P, VCOLS), dtype=_NP_DT)
        for i in ORDER:
            s = SEQS[i]
            nf, rem, n_k = _NF[i], _REM[i], _NK[i]
            aoff = _PAOFF[i]
            boff = aoff + 2 * nf * s
            for hh in range(2):
                h = 2 * c + hh
                blk = batch1[_A[i] + h * s * s: _A[i] + (h + 1) * s * s]
                blkT = blk.reshape(s, s).T.astype(_NP_DT)  # [k, q]
                if nf:
                    po = aoff + hh * nf * s
                    pc[:, po:po + nf * s] = (
                        blkT[:nf * P].reshape(nf, P, s)
                        .transpose(1, 0, 2).reshape(P, nf * s))
                if rem:
                    po = boff + hh * s
                    pc[0:rem, po:po + s] = blkT[nf * P:]
            # V rows for this seq, both local heads, 128-aligned chunks
            vrows = b2[_B[i]:_B[i] + s, 2 * c * EMBED:(2 * c + 2) * EMBED]
            vcol = _VOFF[i] * 2 * EMBED
            vpad = np.zeros((n_k * P, 2 * EMBED), dtype=_NP_DT)
            vpad[:s] = vrows.astype(_NP_DT)
            vc[:, vcol:vcol + n_k * 2 * EMBED] = (
                vpad.reshape(n_k, P, 2 * EMBED)
                .transpose(1, 0, 2).reshape(P, n_k * 2 * EMBED))
        p_cores.append(pc)
        v_cores.append(vc)
    return p_cores, v_cores


def unpack_outputs(o_cores) -> np.ndarray:
    """Scatter per-core packed out^T back to [NTOK, HEADS, EMBED] fp32."""
    out = np.empty((NTOK, HEADS * EMBED), dtype=np.float32)
    for c in range(N_CORES):
        oc = np.asarray(o_cores[c], dtype=np.float32)
        for i in ORDER:
            s = SEQS[i]
            blk = oc[:, _OOFF[i]:_OOFF[i] + s]  # [128, s]
            out[_B[i]:_B[i] + s, 2 * c * EMBED:(2 * c + 1) * EMBED] = \
                blk[0:EMBED].T
            out[_B[i]:_B[i] + s, (2 * c + 1) * EMBED:(2 * c + 2) * EMBED] = \
                blk[EMBED:2 * EMBED].T
    return out.reshape(NTOK, HEADS, EMBED)


# ---------------------------------------------------------------------------
# Execution: run_bass_kernel_spmd over 8 cores (axon/PJRT path).
# ---------------------------------------------------------------------------
_CACHE = {}


def make_in_maps(batch1, batch2):
    p_cores, v_cores = pack_inputs(
        np.asarray(batch1, np.float32), np.asarray(batch2, np.float32))
    return [{"p": p_cores[c], "v": v_cores[c]} for c in range(N_CORES)]


def run_packed(in_maps):
    """Run the SPMD program; returns list of per-core packed outputs."""
    import concourse.bass_utils as bass_utils

    if ("nc", 1) not in _CACHE:
        _CACHE[("nc", 1)] = build_program()
    nc = _CACHE[("nc", 1)]
    res = bass_utils.run_bass_kernel_spmd(nc, in_maps,
                                          core_ids=list(range(N_CORES)))
    return [res.results[c]["o"] for c in range(N_CORES)]


def kernel(batch1, batch2, batch, seqlen) -> np.ndarray:
    in_maps = make_in_maps(batch1, batch2)
    o_cores = run_packed(in_maps)
    return unpack_outputs(o_cores)


# revision 30
# speedup vs baseline: 1.0090x; 1.0040x over previous
"""Trainium2 Bass kernel for ragged bmm2 (attention probs @ V, grouped GEMM).

Problem: 32 ragged sequences, lengths s_i = 128 + 12*i (128..500), 16 heads,
embed 64.  batch1 = packed per-(seq,head) [s,s] prob blocks (fp32, ~227MB),
batch2 = packed V [ntokens, 16*64].  out[q,h,e] = sum_k P[h,q,k] V[k,h,e].

Sharding: head-parallel.  Core c handles heads (2c, 2c+1) for ALL sequences:
identical per-core work/schedule (SPMD-friendly), perfect balance.

v2 design (vs v1): all compute in fp16; P is transposed on the HOST during
packing so the device does zero PE transposes; the matmul streams P^T as the
moving operand against a stationary V chunk, producing out^T [64, s] blocks
per (seq, head) with only n_k matmul instructions per (seq, head); out^T is
un-transposed on the host.  All HBM buffers are packed partition-major so
every DMA is long contiguous runs per partition.

Device per (seq): DMA P^T tiles for both heads, then per (head, k-chunk) one
matmul acc[64h:64h+64, 0:s] += V_chunk[kn,64].T @ PT_chunk[kn, s] accumulated
over k-chunks in PSUM, copy [128, s] fp32->fp16 to an SBUF staging tile, and
one store DMA per group of sequences.
"""

import math

import numpy as np

import bass_rust
import concourse.bass as bass
import concourse.tile as tile
import concourse.mybir as mybir
from concourse.vector_clock import ScopedClock
from concourse.bass2jax import install_neuronx_cc_hook, _bass_exec_p

# ---------------------------------------------------------------------------
# Workarounds for the in-container walrus build, which only accepts a small
# number of sem waits per instruction: split excess waits onto NoOps placed
# immediately before the instruction on the same engine queue.
# ---------------------------------------------------------------------------
MAX_WAITS = 1

_nop_ctr = [0]


def _mk_wait_nop(engine, waits):
    _nop_ctr[0] += 1
    nop = bass_rust.InstNoOp(name=f"I-waitsplit-{_nop_ctr[0]}", ins=[], outs=[],
                             engine=engine)
    nop.sync_info = bass_rust.SyncInfo(on_wait=list(waits), on_update=[])
    return nop


def _split_inst_waits(ordered):
    for bb_name, insts in ordered.items():
        new = []
        for inst in insts:
            si = getattr(inst, "sync_info", None)
            eng = getattr(inst, "engine", None)
            if si is not None and eng is not None:
                waits = list(si.on_wait)
                if len(waits) > MAX_WAITS:
                    extra, keep = waits[:-MAX_WAITS], waits[-MAX_WAITS:]
                    for j in range(0, len(extra), MAX_WAITS):
                        new.append(_mk_wait_nop(eng, extra[j:j + MAX_WAITS]))
                    inst.sync_info = bass_rust.SyncInfo(
                        on_wait=keep, on_update=list(si.on_update))
            new.append(inst)
        insts[:] = new
    return ordered


if not getattr(tile.TileContext, "_waitsplit_patched", False):
    _orig_lower = tile.TileContext._lower_ordered_insts

    def _patched_lower(self, ordered):
        return _orig_lower(self, _split_inst_waits(ordered))

    def _patched_drain_and_barrier(self, tick_clock, wait_clock):
        nc = self.nc
        drain_inst = nc.sync.drain()
        wait_clock.add_sem_waits(
            drain_inst.ins, ScopedClock({None: tick_clock.global_clock}))
        si = drain_inst.ins.sync_info
        waits = list(si.on_wait)
        if len(waits) > MAX_WAITS:
            drain_inst.ins.sync_info = bass_rust.SyncInfo(
                on_wait=waits[:MAX_WAITS], on_update=list(si.on_update))
            for j in range(MAX_WAITS, len(waits), MAX_WAITS):
                nop = nc.sync.nop(nofuse=True)
                nop.ins.sync_info = bass_rust.SyncInfo(
                    on_wait=waits[j:j + MAX_WAITS], on_update=[])
        nc.all_engine_barrier()
        assert self.sems is not None
        popped = nc._tile_sem_poison_stack.pop()
        assert popped is self._sem_poison
        nc.clear_and_free_semaphores(list(self.sems.allocated().values()))
        nc.all_engine_barrier()

    tile.TileContext._lower_ordered_insts = _patched_lower
    tile.TileContext._drain_and_barrier = _patched_drain_and_barrier
    tile.TileContext._waitsplit_patched = True

HEADS = 16
EMBED = 64
BATCH = 32
N_CORES = 8
P = 128  # partitions

SEQS = [128 + 12 * i for i in range(BATCH)]
NTOK = sum(SEQS)  # 10048
# seq start offsets in batch1 (elements) and batch2 (rows)
_A = np.concatenate([[0], np.cumsum([HEADS * s * s for s in SEQS])])
_B = np.concatenate([[0], np.cumsum(SEQS)])
# schedule: interleave shortest/longest so every pipeline window carries a
# balanced DMA-bytes vs PE-work mix; starts with the shortest seq (fast
# pipeline ramp: the first matmul only waits on tiny V/P transfers)
ORDER = []
for _a, _b in zip(range(BATCH // 2), range(BATCH - 1, BATCH // 2 - 1, -1)):
    ORDER += [_a, _b]

_NF = {i: SEQS[i] // P for i in range(BATCH)}           # full k-chunks
_REM = {i: SEQS[i] % P for i in range(BATCH)}           # remainder k rows
_NK = {i: math.ceil(SEQS[i] / P) for i in range(BATCH)}  # total k-chunks

# ---- packed P^T layout: global [128, PCOLS] partition-major -------------
# P^T chunks are grouped into DMA clusters (defined below); each cluster
# occupies a contiguous col span [A region: full 128-row chunks][B region:
# remainder chunks, rows 0..rem].  Partition p = k row within chunk.

# ---- packed V layout: global [128, VCOLS]; per seq 128-row-aligned ------
# chunk kc of seq i at cols (VOFF[i]+kc)*128 .. +128 (cols = head0|head1
# embed, 2*EMBED=128 wide); partition = k row within chunk (zero padded).
_VOFF = {}
_vc = 0
for _i in ORDER:
    _VOFF[_i] = _vc
    _vc += _NK[_i]
VCOLS = _vc * (2 * EMBED)  # 94*128 = 12032

# ---- out^T layout: global [128, OUTCOLS=NTOK] -----------------------------
# per seq block [128, s]: partitions 0:64 = head even out^T [64, s],
# 64:128 = head odd.
_OOFF = {}
_oc = 0
for _i in ORDER:
    _OOFF[_i] = _oc
    _oc += SEQS[_i]
OUTCOLS = _oc  # 10048

# store groups: consecutive runs of ORDER staged in one SBUF tile + 1 DMA
_GSIZES = [4, 8, 8, 8, 4]
GROUPS = []
_g0 = 0
for _gs in _GSIZES:
    GROUPS.append(ORDER[_g0:_g0 + _gs])
    _g0 += _gs
assert _g0 == BATCH

# Per-seq P tile layout [128, 2*n_k*s]: region A = full k-chunks of both
# heads (cols (hh*nf+kc)*s, 128 rows), then region B = remainder chunks
# (cols 2*nf*s + hh*s, rows 0..rem).  A and B are DMAed separately (B with
# rem partition rows => exact transfer bytes), on different issue engines.
_PAOFF = {}  # HBM col offset of seq's region A
_pc = 0
for _i in ORDER:
    _PAOFF[_i] = _pc
    _pc += 2 * _NK[_i] * SEQS[_i]
PCOLS = _pc  # 2*sum(n_k*s) = 64816

COMPUTE_DT = mybir.dt.bfloat16
import ml_dtypes
_NP_DT = ml_dtypes.bfloat16


def build_program(repeat: int = 1):
    """Build the Bass program (one SPMD program shared by all 8 cores)."""
    nc = bass.Bass("TRN2", target_bir_lowering=False, debug=False,
                   num_devices=N_CORES)
    cdt = COMPUTE_DT
    p_d = nc.dram_tensor("p", [P, PCOLS], cdt, kind="ExternalInput").ap()
    v_d = nc.dram_tensor("v", [P, VCOLS], cdt, kind="ExternalInput").ap()
    o_d = nc.dram_tensor("o", [P, OUTCOLS], cdt, kind="ExternalOutput").ap()

    with tile.TileContext(nc) as tc:
        with (
            tc.tile_pool(name="vpool", bufs=len(GROUPS)) as vpool,
            tc.tile_pool(name="ppool", bufs=10) as ppool,
            tc.tile_pool(name="accp", bufs=8, space="PSUM") as acc_pool,
            tc.tile_pool(name="outsb", bufs=2) as out_pool,
        ):
            # V stays resident in SBUF, one tile per store-group so matmuls
            # only depend on their own group's V DMA
            _vgrp = []
            _c0 = 0
            for grp in GROUPS:
                last = grp[-1]
                _c1 = (_VOFF[last] + _NK[last]) * 2 * EMBED
                _vgrp.append((_c0, _c1))
                _c0 = _c1
            vts = {}
            for g in range(len(GROUPS)):
                c0, c1 = _vgrp[g]
                vts[g] = vpool.tile([P, c1 - c0], cdt, name=f"vt{g}", tag="vt")
            nc.sync.dma_start(vts[0][:], v_d[:, _vgrp[0][0]:_vgrp[0][1]])

            for _rep in range(repeat):
              flip = 0
              for g, grp in enumerate(GROUPS):
                vt = vts[g]
                vbase = _vgrp[g][0]
                gbase = _OOFF[grp[0]]
                gcols = sum(SEQS[i] for i in grp)
                osb = out_pool.tile([P, gcols], cdt, tag="osb")
                for si, i in enumerate(grp):
                    if si == 2 and g + 1 < len(GROUPS):
                        c0, c1 = _vgrp[g + 1]
                        nc.scalar.dma_start(vts[g + 1][:], v_d[:, c0:c1])
                    s = SEQS[i]
                    nf, rem, n_k = _NF[i], _REM[i], _NK[i]
                    pt = ppool.tile([P, 2 * n_k * s], cdt,
                                    name=f"pt{i}", tag="pt")
                    poff = _PAOFF[i]
                    ca = 2 * nf * s
                    e0, e1 = ((nc.sync, nc.scalar) if flip == 0
                              else (nc.scalar, nc.sync))
                    flip ^= 1
                    e0.dma_start(pt[:, 0:ca], p_d[:, poff:poff + ca])
                    if rem:
                        e1.dma_start(
                            pt[0:rem, ca:ca + 2 * s],
                            p_d[0:rem, poff + ca:poff + ca + 2 * s])
                    acc = acc_pool.tile([P, s], mybir.dt.float32,
                                        name=f"acc{i}", tag="acc")
                    for hh in range(2):
                        for kc in range(n_k):
                            kn = P if kc < nf else rem
                            vcol = ((_VOFF[i] + kc) * 2 * EMBED
                                    + hh * EMBED - vbase)
                            if kc < nf:
                                pcol = (hh * nf + kc) * s
                            else:
                                pcol = ca + hh * s
                            nc.tensor.matmul(
                                acc[hh * EMBED:(hh + 1) * EMBED, 0:s],
                                lhsT=vt[0:kn, vcol:vcol + EMBED],
                                rhs=pt[0:kn, pcol:pcol + s],
                                start=(kc == 0),
                                stop=(kc == n_k - 1),
                            )
                    ocol = _OOFF[i] - gbase
                    nc.vector.tensor_copy(osb[:, ocol:ocol + s], acc[:])
                    if g == len(GROUPS) - 1:
                        # last group: store per seq so the final transfer
                        # (and hence the kernel tail) is small
                        seng = nc.scalar if si % 2 == 0 else nc.sync
                        seng.dma_start(
                            o_d[:, _OOFF[i]:_OOFF[i] + s],
                            osb[:, ocol:ocol + s])
                if g < len(GROUPS) - 1:
                    seng = nc.scalar if g % 2 == 0 else nc.sync
                    seng.dma_start(o_d[:, gbase:gbase + gcols], osb[:])
    return nc


def pack_inputs(batch1: np.ndarray, batch2: np.ndarray):
    """Build per-core packed (p_core [128, PCOLS], v_core [128, VCOLS])."""
    b2 = np.ascontiguousarray(batch2).reshape(NTOK, HEADS * EMBED)
    p_cores = []
    v_cores = []
    for c in range(N_CORES):
        pc = np.zeros((P, PCOLS), dtype=_NP_DT)
        vc = np.zeros((P, VCOLS), dtype=_NP_DT)
        for i in ORDER:
            s = SEQS[i]
            nf, rem, n_k = _NF[i], _REM[i], _NK[i]
            aoff = _PAOFF[i]
            boff = aoff + 2 * nf * s
            for hh in range(2):
                h = 2 * c + hh
                blk = batch1[_A[i] + h * s * s: _A[i] + (h + 1) * s * s]
                blkT = blk.reshape(s, s).T.astype(_NP_DT)  # [k, q]
                if nf:
                    po = aoff + hh * nf * s
                    pc[:, po:po + nf * s] = (
                        blkT[:nf * P].reshape(nf, P, s)
                        .transpose(1, 0, 2).reshape(P, nf * s))
                if rem:
                    po = boff + hh * s
                    pc[0:rem, po:po + s] = blkT[nf * P:]
            # V rows for this seq, both local heads, 128-aligned chunks
            vrows = b2[_B[i]:_B[i] + s, 2 * c * EMBED:(2 * c + 2) * EMBED]
            vcol = _VOFF[i] * 2 * EMBED
            vpad = np.zeros((n_k * P, 2 * EMBED), dtype=_NP_DT)
            vpad[:s] = vrows.astype(_NP_DT)
            vc[:, vcol:vcol + n_k * 2 * EMBED] = (
                vpad.reshape(n_k, P, 2 * EMBED)
                .transpose(1, 0, 2).reshape(P, n_k * 2 * EMBED))
        p_cores.append(pc)
        v_cores.append(vc)
    return p_cores, v_cores


def unpack_outputs(o_cores) -> np.ndarray:
    """Scatter per-core packed out^T back to [NTOK, HEADS, EMBED] fp32."""
    out = np.empty((NTOK, HEADS * EMBED), dtype=np.float32)
    for c in range(N_CORES):
        oc = np.asarray(o_cores[c], dtype=np.float32)
        for i in ORDER:
            s = SEQS[i]
            blk = oc[:, _OOFF[i]:_OOFF[i] + s]  # [128, s]
            out[_B[i]:_B[i] + s, 2 * c * EMBED:(2 * c + 1) * EMBED] = \
                blk[0:EMBED].T
            out[_B[i]:_B[i] + s, (2 * c + 1) * EMBED:(2 * c + 2) * EMBED] = \
                blk[EMBED:2 * EMBED].T
    return out.reshape(NTOK, HEADS, EMBED)


# ---------------------------------------------------------------------------
# Execution: run_bass_kernel_spmd over 8 cores (axon/PJRT path).
# ---------------------------------------------------------------------------
_CACHE = {}


def make_in_maps(batch1, batch2):
    p_cores, v_cores = pack_inputs(
        np.asarray(batch1, np.float32), np.asarray(batch2, np.float32))
    return [{"p": p_cores[c], "v": v_cores[c]} for c in range(N_CORES)]


def run_packed(in_maps):
    """Run the SPMD program; returns list of per-core packed outputs."""
    import concourse.bass_utils as bass_utils

    if ("nc", 1) not in _CACHE:
        _CACHE[("nc", 1)] = build_program()
    nc = _CACHE[("nc", 1)]
    res = bass_utils.run_bass_kernel_spmd(nc, in_maps,
                                          core_ids=list(range(N_CORES)))
    return [res.results[c]["o"] for c in range(N_CORES)]


def kernel(batch1, batch2, batch, seqlen) -> np.ndarray:
    in_maps = make_in_maps(batch1, batch2)
    o_cores = run_packed(in_maps)
    return unpack_outputs(o_cores)


# revision 35
# speedup vs baseline: 1.0155x; 1.0064x over previous
"""Trainium2 Bass kernel for ragged bmm2 (attention probs @ V, grouped GEMM).

Problem: 32 ragged sequences, lengths s_i = 128 + 12*i (128..500), 16 heads,
embed 64.  batch1 = packed per-(seq,head) [s,s] prob blocks (fp32, ~227MB),
batch2 = packed V [ntokens, 16*64].  out[q,h,e] = sum_k P[h,q,k] V[k,h,e].

Sharding: head-parallel.  Core c handles heads (2c, 2c+1) for ALL sequences:
identical per-core work/schedule (SPMD-friendly), perfect balance.

v2 design (vs v1): all compute in fp16; P is transposed on the HOST during
packing so the device does zero PE transposes; the matmul streams P^T as the
moving operand against a stationary V chunk, producing out^T [64, s] blocks
per (seq, head) with only n_k matmul instructions per (seq, head); out^T is
un-transposed on the host.  All HBM buffers are packed partition-major so
every DMA is long contiguous runs per partition.

Device per (seq): DMA P^T tiles for both heads, then per (head, k-chunk) one
matmul acc[64h:64h+64, 0:s] += V_chunk[kn,64].T @ PT_chunk[kn, s] accumulated
over k-chunks in PSUM, copy [128, s] fp32->fp16 to an SBUF staging tile, and
one store DMA per group of sequences.
"""

import math

import numpy as np

import bass_rust
import concourse.bass as bass
import concourse.tile as tile
import concourse.mybir as mybir
from concourse.vector_clock import ScopedClock
from concourse.bass2jax import install_neuronx_cc_hook, _bass_exec_p

# ---------------------------------------------------------------------------
# Workarounds for the in-container walrus build, which only accepts a small
# number of sem waits per instruction: split excess waits onto NoOps placed
# immediately before the instruction on the same engine queue.
# ---------------------------------------------------------------------------
MAX_WAITS = 1

_nop_ctr = [0]


def _mk_wait_nop(engine, waits):
    _nop_ctr[0] += 1
    nop = bass_rust.InstNoOp(name=f"I-waitsplit-{_nop_ctr[0]}", ins=[], outs=[],
                             engine=engine)
    nop.sync_info = bass_rust.SyncInfo(on_wait=list(waits), on_update=[])
    return nop


def _split_inst_waits(ordered):
    for bb_name, insts in ordered.items():
        new = []
        for inst in insts:
            si = getattr(inst, "sync_info", None)
            eng = getattr(inst, "engine", None)
            if si is not None and eng is not None:
                waits = list(si.on_wait)
                if len(waits) > MAX_WAITS:
                    extra, keep = waits[:-MAX_WAITS], waits[-MAX_WAITS:]
                    for j in range(0, len(extra), MAX_WAITS):
                        new.append(_mk_wait_nop(eng, extra[j:j + MAX_WAITS]))
                    inst.sync_info = bass_rust.SyncInfo(
                        on_wait=keep, on_update=list(si.on_update))
            new.append(inst)
        insts[:] = new
    return ordered


if not getattr(tile.TileContext, "_waitsplit_patched", False):
    _orig_lower = tile.TileContext._lower_ordered_insts

    def _patched_lower(self, ordered):
        return _orig_lower(self, _split_inst_waits(ordered))

    def _patched_drain_and_barrier(self, tick_clock, wait_clock):
        nc = self.nc
        drain_inst = nc.sync.drain()
        wait_clock.add_sem_waits(
            drain_inst.ins, ScopedClock({None: tick_clock.global_clock}))
        si = drain_inst.ins.sync_info
        waits = list(si.on_wait)
        if len(waits) > MAX_WAITS:
            drain_inst.ins.sync_info = bass_rust.SyncInfo(
                on_wait=waits[:MAX_WAITS], on_update=list(si.on_update))
            for j in range(MAX_WAITS, len(waits), MAX_WAITS):
                nop = nc.sync.nop(nofuse=True)
                nop.ins.sync_info = bass_rust.SyncInfo(
                    on_wait=waits[j:j + MAX_WAITS], on_update=[])
        nc.all_engine_barrier()
        assert self.sems is not None
        popped = nc._tile_sem_poison_stack.pop()
        assert popped is self._sem_poison
        nc.clear_and_free_semaphores(list(self.sems.allocated().values()))
        nc.all_engine_barrier()

    tile.TileContext._lower_ordered_insts = _patched_lower
    tile.TileContext._drain_and_barrier = _patched_drain_and_barrier
    tile.TileContext._waitsplit_patched = True

HEADS = 16
EMBED = 64
BATCH = 32
N_CORES = 8
P = 128  # partitions

SEQS = [128 + 12 * i for i in range(BATCH)]
NTOK = sum(SEQS)  # 10048
# seq start offsets in batch1 (elements) and batch2 (rows)
_A = np.concatenate([[0], np.cumsum([HEADS * s * s for s in SEQS])])
_B = np.concatenate([[0], np.cumsum(SEQS)])
# schedule: descending length
ORDER = sorted(range(BATCH), key=lambda i: -SEQS[i])

_NF = {i: SEQS[i] // P for i in range(BATCH)}           # full k-chunks
_REM = {i: SEQS[i] % P for i in range(BATCH)}           # remainder k rows
_NK = {i: math.ceil(SEQS[i] / P) for i in range(BATCH)}  # total k-chunks

# ---- packed P^T layout: global [128, PCOLS] partition-major -------------
# P^T chunks are grouped into DMA clusters (defined below); each cluster
# occupies a contiguous col span [A region: full 128-row chunks][B region:
# remainder chunks, rows 0..rem].  Partition p = k row within chunk.

# ---- packed V layout: global [128, VCOLS]; per seq 128-row-aligned ------
# chunk kc of seq i at cols (VOFF[i]+kc)*128 .. +128 (cols = head0|head1
# embed, 2*EMBED=128 wide); partition = k row within chunk (zero padded).
_VOFF = {}
_vc = 0
for _i in ORDER:
    _VOFF[_i] = _vc
    _vc += _NK[_i]
VCOLS = _vc * (2 * EMBED)  # 94*128 = 12032

# ---- out^T layout: global [128, OUTCOLS=NTOK] -----------------------------
# per seq block [128, s]: partitions 0:64 = head even out^T [64, s],
# 64:128 = head odd.
_OOFF = {}
_oc = 0
for _i in ORDER:
    _OOFF[_i] = _oc
    _oc += SEQS[_i]
OUTCOLS = _oc  # 10048

# store groups: consecutive runs of ORDER staged in one SBUF tile + 1 DMA
GROUP_SIZE = 8
GROUPS = [ORDER[g:g + GROUP_SIZE] for g in range(0, BATCH, GROUP_SIZE)]

# Per-seq P tile layout [128, 2*n_k*s]: region A = full k-chunks of both
# heads (cols (hh*nf+kc)*s, 128 rows), then region B = remainder chunks
# (cols 2*nf*s + hh*s, rows 0..rem).  A and B are DMAed separately (B with
# rem partition rows => exact transfer bytes), on different issue engines.
_PAOFF = {}  # HBM col offset of seq's region A
_pc = 0
for _i in ORDER:
    _PAOFF[_i] = _pc
    _pc += 2 * _NK[_i] * SEQS[_i]
PCOLS = _pc  # 2*sum(n_k*s) = 64816

COMPUTE_DT = mybir.dt.float16
_NP_DT = np.float16


def build_program(repeat: int = 1):
    """Build the Bass program (one SPMD program shared by all 8 cores)."""
    nc = bass.Bass("TRN2", target_bir_lowering=False, debug=False,
                   num_devices=N_CORES)
    cdt = COMPUTE_DT
    p_d = nc.dram_tensor("p", [P, PCOLS], cdt, kind="ExternalInput").ap()
    v_d = nc.dram_tensor("v", [P, VCOLS], cdt, kind="ExternalInput").ap()
    o_d = nc.dram_tensor("o", [P, OUTCOLS], cdt, kind="ExternalOutput").ap()

    with tile.TileContext(nc) as tc:
        with (
            tc.tile_pool(name="vpool", bufs=len(GROUPS)) as vpool,
            tc.tile_pool(name="ppool", bufs=8) as ppool,
            tc.tile_pool(name="accp", bufs=6, space="PSUM") as acc_pool,
            tc.tile_pool(name="outsb", bufs=2) as out_pool,
        ):
            # V stays resident in SBUF, one tile per store-group so matmuls
            # only depend on their own group's V DMA
            _vgrp = []
            _c0 = 0
            for grp in GROUPS:
                last = grp[-1]
                _c1 = (_VOFF[last] + _NK[last]) * 2 * EMBED
                _vgrp.append((_c0, _c1))
                _c0 = _c1
            vts = {}
            for g in range(len(GROUPS)):
                c0, c1 = _vgrp[g]
                vts[g] = vpool.tile([P, c1 - c0], cdt, name=f"vt{g}", tag="vt")
            nc.sync.dma_start(vts[0][:], v_d[:, _vgrp[0][0]:_vgrp[0][1]])

            for _rep in range(repeat):
              flip = 0
              for g, grp in enumerate(GROUPS):
                vt = vts[g]
                vbase = _vgrp[g][0]
                gbase = _OOFF[grp[0]]
                gcols = sum(SEQS[i] for i in grp)
                osb = out_pool.tile([P, gcols], cdt, tag="osb")
                for si, i in enumerate(grp):
                    if si == 2 and g + 1 < len(GROUPS):
                        c0, c1 = _vgrp[g + 1]
                        nc.scalar.dma_start(vts[g + 1][:], v_d[:, c0:c1])
                    s = SEQS[i]
                    nf, rem, n_k = _NF[i], _REM[i], _NK[i]
                    pt = ppool.tile([P, 2 * n_k * s], cdt,
                                    name=f"pt{i}", tag="pt")
                    poff = _PAOFF[i]
                    ca = 2 * nf * s
                    e0, e1 = ((nc.sync, nc.scalar) if flip == 0
                              else (nc.scalar, nc.sync))
                    flip ^= 1
                    e0.dma_start(pt[:, 0:ca], p_d[:, poff:poff + ca])
                    if rem:
                        e1.dma_start(
                            pt[0:rem, ca:ca + 2 * s],
                            p_d[0:rem, poff + ca:poff + ca + 2 * s])
                    acc = acc_pool.tile([P, s], mybir.dt.float32,
                                        name=f"acc{i}", tag="acc")
                    for hh in range(2):
                        for kc in range(n_k):
                            kn = P if kc < nf else rem
                            vcol = ((_VOFF[i] + kc) * 2 * EMBED
                                    + hh * EMBED - vbase)
                            if kc < nf:
                                pcol = (hh * nf + kc) * s
                            else:
                                pcol = ca + hh * s
                            nc.tensor.matmul(
                                acc[hh * EMBED:(hh + 1) * EMBED, 0:s],
                                lhsT=vt[0:kn, vcol:vcol + EMBED],
                                rhs=pt[0:kn, pcol:pcol + s],
                                start=(kc == 0),
                                stop=(kc == n_k - 1),
                            )
                    ocol = _OOFF[i] - gbase
                    nc.vector.tensor_copy(osb[:, ocol:ocol + s], acc[:])
                seng = nc.scalar if g % 2 == 0 else nc.sync
                seng.dma_start(o_d[:, gbase:gbase + gcols], osb[:])
    return nc


def pack_inputs(batch1: np.ndarray, batch2: np.ndarray):
    """Build per-core packed (p_core [128, PCOLS], v_core [128, VCOLS])."""
    b2 = np.ascontiguousarray(batch2).reshape(NTOK, HEADS * EMBED)
    p_cores = []
    v_cores = []
    for c in range(N_CORES):
        pc = np.zeros((P, PCOLS), dtype=_NP_DT)
        vc = np.zeros((P, VCOLS), dtype=_NP_DT)
        for i in ORDER:
            s = SEQS[i]
            nf, rem, n_k = _NF[i], _REM[i], _NK[i]
            aoff = _PAOFF[i]
            boff = aoff + 2 * nf * s
            for hh in range(2):
                h = 2 * c + hh
                blk = batch1[_A[i] + h * s * s: _A[i] + (h + 1) * s * s]
                blkT = blk.reshape(s, s).T.astype(_NP_DT)  # [k, q]
                if nf:
                    po = aoff + hh * nf * s
                    pc[:, po:po + nf * s] = (
                        blkT[:nf * P].reshape(nf, P, s)
                        .transpose(1, 0, 2).reshape(P, nf * s))
                if rem:
                    po = boff + hh * s
                    pc[0:rem, po:po + s] = blkT[nf * P:]
            # V rows for this seq, both local heads, 128-aligned chunks
            vrows = b2[_B[i]:_B[i] + s, 2 * c * EMBED:(2 * c + 2) * EMBED]
            vcol = _VOFF[i] * 2 * EMBED
            vpad = np.zeros((n_k * P, 2 * EMBED), dtype=_NP_DT)
            vpad[:s] = vrows.astype(_NP_DT)
            vc[:, vcol:vcol + n_k * 2 * EMBED] = (
                vpad.reshape(n_k, P, 2 * EMBED)
                .transpose(1, 0, 2).reshape(P, n_k * 2 * EMBED))
        p_cores.append(pc)
        v_cores.append(vc)
    return p_cores, v_cores


def unpack_outputs(o_cores) -> np.ndarray:
    """Scatter per-core packed out^T back to [NTOK, HEADS, EMBED] fp32."""
    out = np.empty((NTOK, HEADS * EMBED), dtype=np.float32)
    for c in range(N_CORES):
        oc = np.asarray(o_cores[c], dtype=np.float32)
        for i in ORDER:
            s = SEQS[i]
            blk = oc[:, _OOFF[i]:_OOFF[i] + s]  # [128, s]
            out[_B[i]:_B[i] + s, 2 * c * EMBED:(2 * c + 1) * EMBED] = \
                blk[0:EMBED].T
            out[_B[i]:_B[i] + s, (2 * c + 1) * EMBED:(2 * c + 2) * EMBED] = \
                blk[EMBED:2 * EMBED].T
    return out.reshape(NTOK, HEADS, EMBED)


# ---------------------------------------------------------------------------
# Execution: run_bass_kernel_spmd over 8 cores (axon/PJRT path).
# ---------------------------------------------------------------------------
_CACHE = {}


def make_in_maps(batch1, batch2):
    p_cores, v_cores = pack_inputs(
        np.asarray(batch1, np.float32), np.asarray(batch2, np.float32))
    return [{"p": p_cores[c], "v": v_cores[c]} for c in range(N_CORES)]


def run_packed(in_maps):
    """Run the SPMD program; returns list of per-core packed outputs."""
    import concourse.bass_utils as bass_utils

    if ("nc", 1) not in _CACHE:
        _CACHE[("nc", 1)] = build_program()
    nc = _CACHE[("nc", 1)]
    res = bass_utils.run_bass_kernel_spmd(nc, in_maps,
                                          core_ids=list(range(N_CORES)))
    return [res.results[c]["o"] for c in range(N_CORES)]


def kernel(batch1, batch2, batch, seqlen) -> np.ndarray:
    in_maps = make_in_maps(batch1, batch2)
    o_cores = run_packed(in_maps)
    return unpack_outputs(o_cores)
